# revision 1
# baseline (speedup 1.0000x reference)
"""Causal multi-head attention (B=2, T=2048, D=1024, H=16) on 8 trn2 cores.

Sharding: data-parallel over batch (2) x tensor-parallel over heads (4 groups
of 4 heads): core c handles batch c//4, head group c%4. Each core computes
q/k/v projections for its 256 feature columns, causal attention for its 4
heads, and a partial row-parallel output projection. The host sums the 4
partials per batch and adds bo.
"""

import sys

if "/opt/trn_rl_repo" not in sys.path:
    sys.path.insert(0, "/opt/trn_rl_repo")

import numpy as np

import concourse.bass as bass
import concourse.mybir as mybir
import concourse.tile as tile
from concourse import bacc

F32 = mybir.dt.float32
F32R = mybir.dt.float32r
EXP = mybir.ActivationFunctionType.Exp

B, T, D, H, HD = 2, 2048, 1024, 16, 64
SCALE = float(D) ** -0.5
NCORES = 8
HPC = 4  # heads per core
JS = HPC * HD  # 256 feature columns per core
NT = T // 128  # 16 t-chunks
ND = D // 128  # 8 d-chunks
NG = T // 512  # 4 query groups
MASKVAL = -1e30

_CACHE = {}

# ablation flags
FLAGS = {"interleave_qk": True, "f32r_transpose": True, "phases": "full"}


def _emit_consts(nc, consts, dram):
    c = {}
    c["ident"] = consts.tile([128, 128], F32, name="ident")
    nc.gpsimd.memset(c["ident"], 0.0)
    nc.gpsimd.affine_select(
        out=c["ident"], in_=c["ident"], compare_op=mybir.AluOpType.not_equal,
        fill=1.0, base=0, pattern=[[-1, 128]], channel_multiplier=1,
    )
    # causal mask band: M[p, s] = 0 if s >= p + 512 else MASKVAL
    # slice M[:, 512-junk:] is a [p, f]-mask: 0 iff f >= p + junk
    c["mband"] = consts.tile([128, 640], F32, name="mband")
    nc.gpsimd.memset(c["mband"], 0.0)
    nc.gpsimd.affine_select(
        out=c["mband"], in_=c["mband"], compare_op=mybir.AluOpType.is_ge,
        fill=MASKVAL, base=-512, pattern=[[1, 640]], channel_multiplier=-1,
    )
    c["wq"] = consts.tile([128, ND, JS], F32R, name="wq_sb")
    c["wk"] = consts.tile([128, ND, JS], F32R, name="wk_sb")
    c["wv"] = consts.tile([128, ND, JS], F32R, name="wv_sb")
    for key in ("wq", "wk", "wv"):
        nc.sync.dma_start(
            out=c[key], in_=dram[key].ap().rearrange("(c p) j -> p c j", p=128)
        )
    c["wo"] = consts.tile([128, 2, D], F32R, name="wo_sb")
    nc.sync.dma_start(
        out=c["wo"], in_=dram["wo"].ap().rearrange("(c p) n -> p c n", p=128)
    )
    c["bq"] = consts.tile([128, 2], F32, name="bq_sb")
    c["bk"] = consts.tile([128, 2], F32, name="bk_sb")
    nc.sync.dma_start(out=c["bq"], in_=dram["bq"].ap().rearrange("(c p) -> p c", p=128))
    nc.sync.dma_start(out=c["bk"], in_=dram["bk"].ap().rearrange("(c p) -> p c", p=128))
    c["bv"] = consts.tile([128, JS], F32, name="bv_bc")
    nc.gpsimd.dma_start(
        out=c["bv"], in_=bass.AP(tensor=dram["bv"], offset=0, ap=[[0, 128], [1, JS]])
    )
    c["ones"] = consts.tile([128, HPC * NT], F32, name="ones_sb")
    nc.vector.memset(c["ones"], 1.0)
    c["identr"] = consts.tile([128, 128], F32R, name="identr")
    nc.vector.tensor_copy(c["identr"], c["ident"])
    c["mbandr"] = consts.tile([128, 128], F32R, name="mbandr")
    nc.vector.tensor_copy(c["mbandr"], c["mband"][:, 512:640])
    c["onesr"] = consts.tile([1, 64], F32R, name="onesr")
    nc.vector.tensor_copy(c["onesr"], c["ones"][0:1, 0:64])
    return c


def _emit_body(nc, tc, c, persist, dram, rep):
    """One full attention pass (phases 0-3)."""
    r = f"r{rep}"
    qT = persist["qT"]
    kT = persist["kT"]
    vv = persist["vv"]
    oT = persist["oT"]
    x_d = dram["x"]
    out_d = dram["out"]

    def emit_qk(jc, w_sb, b_sb, dstT, pool, tag, width):
        for tg in range(NG):
            ps = pool.tile([128, width], F32, name=f"ps{tag}{r}_{jc}_{tg}", tag=tag)
            for dc in range(ND):
                nc.tensor.matmul(
                    ps[:, :512],
                    w_sb[:, dc, jc * 128:(jc + 1) * 128],
                    xT[:, dc, tg * 512:(tg + 1) * 512],
                    start=(dc == 0),
                    stop=(dc == ND - 1),
                )
            nc.vector.tensor_scalar_add(
                out=dstT[:, jc, tg * 512:(tg + 1) * 512],
                in0=ps[:, :512],
                scalar1=b_sb[:, jc:jc + 1],
            )

    with tc.tile_pool(name=f"xTpool{r}", bufs=1) as xTpool:
        xT = xTpool.tile([128, ND, T], F32R, name=f"xT{r}")
        # ------------ phase 0: transpose x; phase 1a: k0, q0, v ----------
        with (
            tc.tile_pool(name=f"xstage{r}", bufs=3) as xstage,
            tc.tile_pool(name=f"psA{r}", bufs=2, space="PSUM") as psA,
            tc.tile_pool(name=f"psB{r}", bufs=3, space="PSUM") as psB,
        ):
            for i in range(NT):
                x_sb = xstage.tile([128, D], F32R if FLAGS["f32r_transpose"] else F32, name=f"x_sb{r}", tag="x_sb")
                nc.scalar.dma_start(out=x_sb, in_=x_d.ap()[i * 128:(i + 1) * 128, :])
                for half in range(2):
                    tp = psA.tile([128, 512], F32R if FLAGS["f32r_transpose"] else F32, name=f"tp{r}", tag="tp")
                    for q4 in range(4):
                        dc = half * 4 + q4
                        nc.tensor.transpose(
                            tp[:, q4 * 128:(q4 + 1) * 128],
                            x_sb[:, dc * 128:(dc + 1) * 128],
                            c["identr" if FLAGS["f32r_transpose"] else "ident"],
                        )
                    dst = xT[:, half * 4:(half + 1) * 4, i * 128:(i + 1) * 128]
                    src = tp.rearrange("p (c f) -> p c f", c=4)
                    if half == 0:
                        nc.vector.tensor_copy(dst, src)
                    else:
                        nc.scalar.copy(dst, src)
                # v for this t-chunk (feeds on all 8 d-chunks just written)
                ps = psB.tile([128, 512], F32, name=f"psv{r}", tag="psb")
                for dc in range(ND):
                    nc.tensor.matmul(
                        ps[:, :JS],
                        xT[:, dc, i * 128:(i + 1) * 128],
                        c["wv"][:, dc, :],
                        start=(dc == 0),
                        stop=(dc == ND - 1),
                    )
                nc.vector.tensor_add(
                    out=vv[:, :, i, 0:HD],
                    in0=ps[:, :JS].rearrange("p (h c) -> p h c", h=HPC),
                    in1=c["bv"].rearrange("p (h c) -> p h c", h=HPC),
                )
                # q/k projections for the completed 512-wide t-group
                if i % 4 == 3:
                    tg = i // 4
                    for jc in range(2):
                        for w_sb, b_sb, dstT in (
                            (c["wk"], c["bk"], kT), (c["wq"], c["bq"], qT)
                        ):
                            ps = psB.tile(
                                [128, 512], F32, name=f"psqk{r}", tag="psb"
                            )
                            for dc in range(ND):
                                nc.tensor.matmul(
                                    ps,
                                    w_sb[:, dc, jc * 128:(jc + 1) * 128],
                                    xT[:, dc, tg * 512:(tg + 1) * 512],
                                    start=(dc == 0),
                                    stop=(dc == ND - 1),
                                )
                            nc.vector.tensor_scalar_add(
                                out=dstT[:, jc, tg * 512:(tg + 1) * 512],
                                in0=ps,
                                scalar1=b_sb[:, jc:jc + 1],
                            )
            nc.vector.tensor_copy(
                vv[:, :, :, HD:HD + 1],
                c["ones"].rearrange("p (h i o) -> p h i o", h=HPC, o=1),
            )

        # ------------ phase 2: attention (qk jc1 interleaved) ------------
        if FLAGS["phases"] == "01":
            nc.sync.dma_start(out=out_d.ap()[0:512, :].bitcast(F32R), in_=qT)
            nc.sync.dma_start(out=out_d.ap()[512:1024, :].bitcast(F32R), in_=kT)
            nc.sync.dma_start(out=out_d.ap()[1024:1544, :].bitcast(F32R), in_=vv)
            return
        with (
            tc.tile_pool(name=f"psS{r}", bufs=2, space="PSUM") as psS,
            tc.tile_pool(name=f"psO{r}", bufs=4, space="PSUM") as psO,
            tc.tile_pool(name=f"esb{r}", bufs=7) as esb,
            tc.tile_pool(name=f"nrm{r}", bufs=3) as nrm,
        ):
            def emit_head(h):
                jc, hr = h // 2, (h % 2) * 64
                accs = [
                    psO.tile([128, 512], F32, tag="oacc", name=f"oacc{r}_{h}_{g}")
                    for g in range(NG)
                ]
                pieces = []
                for ck in range(NT):
                    g0 = ck // 4
                    junk = ck * 128 - g0 * 512
                    for pg in range(g0, NG, 2):
                        pieces.append((ck, g0, junk, pg, min(2, NG - pg)))

                def emit_pv(piece, es):
                    ck, g0, junk, pg, pn = piece
                    lo = junk if pg == g0 else 0
                    for gi in range(pn):
                        g = pg + gi
                        glo = lo if gi == 0 else 0
                        nc.tensor.matmul(
                            accs[g][0:HD + 1, glo:512],
                            vv[:, h, ck, :],
                            es[:, gi * 512 + glo:(gi + 1) * 512],
                            start=(ck == 0),
                            stop=(ck == 4 * g + 3),
                        )

                def emit_norm(g):
                    # reciprocal of the denominator row, broadcast via a
                    # K=1 PE outer-product into the free rows of the acc
                    # bank, then one DVE multiply into oT
                    rc = nrm.tile([1, 512], F32, tag="rc", name=f"rc{r}_{h}_{g}")
                    nc.vector.reciprocal(rc, accs[g][HD:HD + 1, :])
                    rb = nrm.tile([64, 512], F32, tag="rb", name=f"rb{r}_{h}_{g}")
                    nc.gpsimd.partition_broadcast(rb, rc)
                    nc.vector.tensor_mul(
                        oT[hr:hr + 64, jc, g * 512:(g + 1) * 512],
                        accs[g][0:HD, :],
                        rb,
                    )

                pending = []  # (piece, es) awaiting PV emission
                done_g = set()

                def flush_one():
                    piece, es = pending.pop(0)
                    emit_pv(piece, es)
                    ck, g0, junk, pg, pn = piece
                    for gi in range(pn):
                        g = pg + gi
                        if ck == 4 * g + 3 and g not in done_g:
                            done_g.add(g)
                            emit_norm(g)

                for piece in pieces:
                    ck, g0, junk, pg, pn = piece
                    lo = junk if pg == g0 else 0
                    ps = psS.tile([128, 1024], F32, name=f"psrow{r}", tag="psrow")
                    for gi in range(pn):
                        g = pg + gi
                        if gi == 0 and pg == g0:
                            # pre-load the additive causal mask for the
                            # diagonal 128-block, then accumulate scores
                            nc.tensor.matmul(
                                ps[:, junk:junk + 128],
                                c["identr"],
                                c["mbandr"],
                                start=True,
                                stop=False,
                            )
                            nc.tensor.matmul(
                                ps[:, gi * 512:(gi + 1) * 512],
                                kT[hr:hr + 64, jc, ck * 128:(ck + 1) * 128],
                                qT[hr:hr + 64, jc, g * 512:(g + 1) * 512],
                                start=False,
                                stop=True,
                            )
                        else:
                            nc.tensor.matmul(
                                ps[:, gi * 512:(gi + 1) * 512],
                                kT[hr:hr + 64, jc, ck * 128:(ck + 1) * 128],
                                qT[hr:hr + 64, jc, g * 512:(g + 1) * 512],
                                start=True,
                                stop=True,
                            )
                    es = esb.tile([128, 1024], F32R, name=f"es{r}", tag="es")
                    nc.scalar.activation(
                        es[:, lo:pn * 512], ps[:, lo:pn * 512], EXP, scale=SCALE
                    )
                    pending.append((piece, es))
                    if len(pending) > FLAGS.get("pv_lag", 2):
                        flush_one()
                while pending:
                    flush_one()

            for h in range(HPC):
                emit_head(h)

    if FLAGS["phases"] == "012":
        nc.sync.dma_start(out=out_d.ap()[0:512, :].bitcast(F32R), in_=oT)
        return
    # ---------------- phase 3: output projection -------------------------
    with (
        tc.tile_pool(name=f"ps3{r}", bufs=4, space="PSUM") as ps3,
        tc.tile_pool(name=f"osb{r}", bufs=4) as osb,
    ):
        for i in range(NT):
            for ng in range(2):
                ps = ps3.tile([128, 512], F32, name=f"ps3t{r}", tag="ps3t")
                for jc in range(2):
                    nc.tensor.matmul(
                        ps,
                        oT[:, jc, i * 128:(i + 1) * 128],
                        c["wo"][:, jc, ng * 512:(ng + 1) * 512],
                        start=(jc == 0),
                        stop=(jc == 1),
                    )
                ob = osb.tile([128, 512], F32, name=f"ob{r}", tag="ob")
                if (i + ng) % 2 == 0:
                    nc.vector.tensor_copy(ob, ps)
                else:
                    nc.scalar.copy(ob, ps)
                nc.sync.dma_start(
                    out=out_d.ap()[
                        i * 128:(i + 1) * 128, ng * 512:(ng + 1) * 512
                    ],
                    in_=ob,
                )


def build(reps=1):
    nc = bacc.Bacc("TRN2", target_bir_lowering=False, num_devices=NCORES)
    dram = {
        "x": nc.dram_tensor(
            "x", [T, D], F32R if FLAGS["f32r_transpose"] else F32,
            kind="ExternalInput",
        ),
        "wq": nc.dram_tensor("wq", [D, JS], F32R, kind="ExternalInput"),
        "wk": nc.dram_tensor("wk", [D, JS], F32R, kind="ExternalInput"),
        "wv": nc.dram_tensor("wv", [D, JS], F32R, kind="ExternalInput"),
        "bq": nc.dram_tensor("bq", [JS], F32, kind="ExternalInput"),
        "bk": nc.dram_tensor("bk", [JS], F32, kind="ExternalInput"),
        "bv": nc.dram_tensor("bv", [JS], F32, kind="ExternalInput"),
        "wo": nc.dram_tensor("wo", [JS, D], F32R, kind="ExternalInput"),
        "out": nc.dram_tensor("out", [T, D], F32, kind="ExternalOutput"),
    }
    with tile.TileContext(nc) as tc:
        with (
            tc.tile_pool(name="consts", bufs=1) as consts,
            tc.tile_pool(name="persist", bufs=1) as persist_pool,
        ):
            c = _emit_consts(nc, consts, dram)
            persist = {
                "qT": persist_pool.tile([128, 2, T], F32R, name="qT"),
                "kT": persist_pool.tile([128, 2, T], F32R, name="kT"),
                "vv": persist_pool.tile([128, HPC, NT, HD + 1], F32R, name="vv"),
                "oT": persist_pool.tile([128, 2, T], F32R, name="oT"),
            }
            for rep in range(reps):
                _emit_body(nc, tc, c, persist, dram, rep)
    nc.compile()
    return nc


def _in_maps(inputs):
    x = np.ascontiguousarray(np.asarray(inputs["x"], dtype=np.float32))
    maps = []
    for cc in range(NCORES):
        b, g = cc // HPC, cc % HPC
        js = slice(g * JS, (g + 1) * JS)
        maps.append(
            {
                "x": np.ascontiguousarray(x[b]),
                "wq": np.ascontiguousarray(np.asarray(inputs["wq"], np.float32)[:, js]),
                "wk": np.ascontiguousarray(np.asarray(inputs["wk"], np.float32)[:, js]),
                "wv": np.ascontiguousarray(np.asarray(inputs["wv"], np.float32)[:, js]),
                "bq": np.ascontiguousarray(np.asarray(inputs["bq"], np.float32)[js]),
                "bk": np.ascontiguousarray(np.asarray(inputs["bk"], np.float32)[js]),
                "bv": np.ascontiguousarray(np.asarray(inputs["bv"], np.float32)[js]),
                "wo": np.ascontiguousarray(np.asarray(inputs["wo"], np.float32)[js, :]),
            }
        )
    return maps


def kernel(**inputs) -> np.ndarray:
    from concourse.bass_utils import run_bass_kernel_spmd

    if "nc" not in _CACHE:
        _CACHE["nc"] = build()
    nc = _CACHE["nc"]
    maps = _in_maps(inputs)
    res = run_bass_kernel_spmd(nc, maps, core_ids=list(range(NCORES)))
    out = np.zeros((B, T, D), dtype=np.float32)
    for cc in range(NCORES):
        out[cc // HPC] += res.results[cc]["out"]
    out += np.asarray(inputs["bo"], np.float32)[None, None, :]
    return out



# revision 22
# speedup vs baseline: 1.2529x; 1.2529x over previous
"""Causal multi-head attention (B=2, T=2048, D=1024, H=16) on 8 trn2 cores.

Sharding: data-parallel over batch (2) x tensor-parallel over heads (4 groups
of 4 heads): core c handles batch c//4, head group c%4. Each core computes
q/k/v projections for its 256 feature columns, causal attention for its 4
heads, and a partial row-parallel output projection. The host sums the 4
partials per batch and adds bo.

Numerics/layout strategy:
- Host pre-transposes x to d-major and pre-casts: xT bf16 (value path) and
  xT8 fp8-e4m3 (q/k path), so the device does zero transposes.
- Q/K projections run as fp8 DoubleRow matmuls (0.5 cycles/row, 256-deep
  contraction). Weights are pre-scaled by 8 on the host (folded back out of
  the softmax exp scale) to keep fp8 away from the subnormal range. q/k are
  stored bf16; QK^T scores and everything downstream run bf16->fp32-psum.
- Attention runs per head over group-pairs (q-cols 0:1024 then 1024:2048)
  with the output projection for finished t-chunks interleaved between
  pairs; persistent state is parity-double-buffered so consecutive reps
  pipeline.
"""

import sys

if "/opt/trn_rl_repo" not in sys.path:
    sys.path.insert(0, "/opt/trn_rl_repo")

import numpy as np
import ml_dtypes

import concourse.bass as bass
import concourse.mybir as mybir
import concourse.tile as tile
from concourse import bacc

F32 = mybir.dt.float32
BF16 = mybir.dt.bfloat16
F8 = mybir.dt.float8e4
EXP = mybir.ActivationFunctionType.Exp
DR = mybir.MatmulPerfMode.DoubleRow

B, T, D, H, HD = 2, 2048, 1024, 16, 64
SCALE = float(D) ** -0.5  # module scales by d_model^-0.5
NCORES = 8
HPC = 4  # heads per core
JS = HPC * HD  # 256 feature columns per core
NT = T // 128  # 16 t-chunks
ND = D // 128  # 8 d-chunks
NG = T // 512  # 4 query groups
WS = 8.0  # fp8 weight prescale, folded out of the exp scale
SCALE_EXP = SCALE / (WS * WS)
MASKVAL = -1e30

NP_BF16 = ml_dtypes.bfloat16
NP_F8 = ml_dtypes.float8_e4m3

_CACHE = {}

# SCORES_DR: store q/k as fp8 in a per-head [32-partition, 2, T] layout and
# run QK^T as fp8 DoubleRow (0.5 cycles/row). Adds ~0.9e-2 of logit noise
# from the fp8 re-quantization of q/k.
SCORES_DR = True

# feature permutation used when SCORES_DR: f' = jc*128 + d2*64 + hh*32 + dl
# <- f = h*64 + d, h = jc*2 + hh, d = d2*32 + dl. The q/k projection PSUM
# partitions then come out as [d2][hh][dl], so two contiguous 64-partition
# DVE copies land q/k straight into the per-head [32, 2, T] DoubleRow
# layout (partition = h*32 + dl, free dim1 = d2).
_jj = np.arange(JS)
_jc, _r = _jj // 128, _jj % 128
_d2, _r2 = _r // 64, _r % 64
_hh, _dl = _r2 // 32, _r2 % 32
PERM = (_jc * 2 + _hh) * 64 + _d2 * 32 + _dl


def _emit_consts(nc, consts, dram):
    c = {}
    ident = consts.tile([128, 128], F32, name="ident")
    nc.gpsimd.memset(ident, 0.0)
    nc.gpsimd.affine_select(
        out=ident, in_=ident, compare_op=mybir.AluOpType.not_equal,
        fill=1.0, base=0, pattern=[[-1, 128]], channel_multiplier=1,
    )
    # diag-block additive causal mask: M[p, j] = 0 if j >= p else -1e30
    mband = consts.tile([128, 128], F32, name="mband")
    nc.gpsimd.memset(mband, 0.0)
    nc.gpsimd.affine_select(
        out=mband, in_=mband, compare_op=mybir.AluOpType.is_ge,
        fill=MASKVAL, base=0, pattern=[[1, 128]], channel_multiplier=-1,
    )
    c["identb"] = consts.tile([128, 128], BF16, name="identb")
    nc.vector.tensor_copy(c["identb"], ident)
    c["mb16"] = consts.tile([128, 128], BF16, name="mb16")
    nc.vector.tensor_copy(c["mb16"], mband)

    for key, shape, dt in (
        ("wq8", [128, 4, 2, JS], F8),
        ("wk8", [128, 4, 2, JS], F8),
        ("wq8lo", [128, 4, 2, JS], F8),
        ("wk8lo", [128, 4, 2, JS], F8),
        ("wv", [128, ND, JS], BF16),
        ("wo", [128, 2, D], BF16),
        ("bq", [128, 2], F32),
        ("bk", [128, 2], F32),
    ):
        c[key] = consts.tile(shape, dt, name=key + "_sb")
        nc.sync.dma_start(out=c[key], in_=dram[key].ap())
    c["bv"] = consts.tile([128, JS], F32, name="bv_bc")
    nc.gpsimd.dma_start(
        out=c["bv"], in_=bass.AP(tensor=dram["bv"], offset=0, ap=[[0, 128], [1, JS]])
    )
    return c


def _emit_proj_tg(nc, c, P, pools, dram, rep, tg):
    """Projections for one 512-wide t-group: v (bf16), q/k (fp8 DoubleRow)."""
    par = rep % 2
    qT, kT, vv = P[par]["qT"], P[par]["kT"], P[par]["vv"]
    xp, x8p, psP = pools["xt"], pools["x8"], pools["psP"]
    r = f"r{rep}"
    ts = slice(tg * 512, (tg + 1) * 512)

    xt = xp.tile([128, ND, 512], BF16, name=f"xt{r}_{tg}", tag="xt")
    nc.scalar.dma_start(out=xt, in_=dram["xT"].ap()[:, :, ts])
    x8 = x8p.tile([128, 4, 2, 512], F8, name=f"x8{r}_{tg}", tag="x8")
    nc.scalar.dma_start(out=x8, in_=dram["xT8"].ap()[:, :, :, ts])

    for i4 in range(4):
        i = tg * 4 + i4
        psv = psP.tile([128, 512], F32, name=f"psv{r}_{i}", tag="pp")
        for dc in range(ND):
            nc.tensor.matmul(
                psv[:, :JS],
                xt[:, dc, i4 * 128:(i4 + 1) * 128],
                c["wv"][:, dc, :],
                start=(dc == 0),
                stop=(dc == ND - 1),
            )
        nc.vector.tensor_add(
            out=vv[:, :, i, 0:HD],
            in0=psv[:, :JS].rearrange("p (h e) -> p h e", h=HPC),
            in1=c["bv"].rearrange("p (h e) -> p h e", h=HPC),
        )
    for w8, w8lo, b_sb, dstT in (
        (c["wk8"], c["wk8lo"], c["bk"], kT),
        (c["wq8"], c["wq8lo"], c["bq"], qT),
    ):
        for jc in range(2):
            ps = psP.tile([128, 512], F32, name=f"psqk{r}_{tg}", tag="pp")
            for c2 in range(4):
                for wi, w_ in enumerate((w8, w8lo)):
                    nc.tensor.matmul(
                        ps,
                        w_[:, c2, :, jc * 128:(jc + 1) * 128],
                        x8[:, c2, :, :],
                        start=(c2 == 0 and wi == 0),
                        stop=(c2 == 3 and wi == 1),
                        perf_mode=DR,
                    )
            if SCORES_DR:
                for d2 in range(2):
                    nc.vector.tensor_scalar_add(
                        out=dstT[jc * 64:(jc + 1) * 64, d2, ts],
                        in0=ps[d2 * 64:(d2 + 1) * 64, :],
                        scalar1=b_sb[d2 * 64:(d2 + 1) * 64, jc:jc + 1],
                    )
            else:
                nc.vector.tensor_scalar_add(
                    out=dstT[:, jc, ts],
                    in0=ps,
                    scalar1=b_sb[:, jc:jc + 1],
                )


def _emit_head_gpair(nc, c, P, pools, rep, h, gset):
    """Scores (fp8 DoubleRow) + exp + p@v + normalize for one head over a
    pair of 512-wide query groups."""
    par = rep % 2
    qT, kT, vv, oT = (P[par][k] for k in ("qT", "kT", "vv", "oT"))
    psS, psA, esb, nrm = pools["psS"], pools["psA"], pools["es"], pools["nrm"]
    r = f"r{rep}"
    jc, hr = h // 2, (h % 2) * 64
    hb = h * 32

    def qk_ap(t, lo_t, n_t):
        if SCORES_DR:
            return t[hb:hb + 32, :, lo_t:lo_t + n_t]
        return t[hr:hr + 64, jc, lo_t:lo_t + n_t]

    mm_kw = {"perf_mode": DR} if SCORES_DR else {}

    accs = {
        g: psA.tile([128, 512], F32, name=f"acc{r}_{h}_{g}", tag="acc")
        for g in gset
    }
    pieces = []
    for ck in range(gset[-1] * 4 + 4):
        glist = [g for g in gset if ck <= 4 * g + 3]
        pieces.append((ck, glist))

    def emit_pv(piece, es):
        ck, glist = piece
        for gi, g in enumerate(glist):
            junk = ck * 128 - g * 512
            glo = junk if junk > 0 else 0
            nc.tensor.matmul(
                accs[g][0:HD + 1, glo:512],
                vv[:, h, ck, 0:HD + 1],
                es[:, gi * 512 + glo:(gi + 1) * 512],
                start=(ck == 0),
                stop=(ck == 4 * g + 3),
            )

    def emit_norm(g):
        rc = nrm.tile([1, 512], F32, name=f"rc{r}_{h}_{g}", tag="rc")
        nc.vector.reciprocal(rc, accs[g][HD:HD + 1, :])
        rb = nrm.tile([64, 512], F32, name=f"rb{r}_{h}_{g}", tag="rb")
        nc.gpsimd.partition_broadcast(rb, rc)
        nc.vector.tensor_mul(
            oT[hr:hr + 64, jc, g * 512:(g + 1) * 512], accs[g][0:HD, :], rb
        )

    pending = []
    done_g = set()

    def flush_one():
        piece, es = pending.pop(0)
        emit_pv(piece, es)
        ck, glist = piece
        for g in glist:
            if ck == 4 * g + 3 and g not in done_g:
                done_g.add(g)
                emit_norm(g)

    for ck, glist in pieces:
        width = len(glist) * 512
        ps = psS.tile([128, width], F32, name=f"psrow{r}_{h}", tag="ps")
        lo = 0
        for gi, g in enumerate(glist):
            junk = ck * 128 - g * 512
            diag = junk >= 0  # only ever at gi == 0
            kslice = qk_ap(kT, ck * 128, 128)
            if diag:
                lo = junk
                # mask preload + scores accumulated onto it for the diagonal
                # 128 block; remaining (fully-causal) columns start a fresh
                # psum group. Junk columns [0:junk) are never computed.
                nc.tensor.matmul(
                    ps[:, junk:junk + 128],
                    c["identb"],
                    c["mb16"],
                    start=True,
                    stop=False,
                )
                nc.tensor.matmul(
                    ps[:, junk:junk + 128],
                    kslice,
                    qk_ap(qT, g * 512 + junk, 128),
                    start=False,
                    stop=True,
                    **mm_kw,
                )
                if junk < 384:
                    nc.tensor.matmul(
                        ps[:, junk + 128:512],
                        kslice,
                        qk_ap(qT, g * 512 + junk + 128, 384 - junk),
                        start=True,
                        stop=True,
                        **mm_kw,
                    )
            else:
                nc.tensor.matmul(
                    ps[:, gi * 512:(gi + 1) * 512],
                    kslice,
                    qk_ap(qT, g * 512, 512),
                    start=True,
                    stop=True,
                    **mm_kw,
                )
        es = esb.tile([128, 1024], BF16, name=f"es{r}_{h}", tag="es")
        nc.scalar.activation(es[:, lo:width], ps[:, lo:width], EXP, scale=SCALE_EXP)
        pending.append(((ck, glist), es))
        if len(pending) > 2:
            flush_one()
    while pending:
        flush_one()


def _emit_wo(nc, c, P, pools, dram, rep, irange):
    """Output projection + store for finished 128-row t-chunks."""
    par = rep % 2
    oT = P[par]["oT"]
    psP, obp = pools["psP"], pools["ob"]
    r = f"r{rep}"
    for i in irange:
        for ng in range(2):
            ps = psP.tile([128, 512], F32, name=f"ps3{r}_{i}", tag="pp")
            for jc in range(2):
                nc.tensor.matmul(
                    ps,
                    oT[:, jc, i * 128:(i + 1) * 128],
                    c["wo"][:, jc, ng * 512:(ng + 1) * 512],
                    start=(jc == 0),
                    stop=(jc == 1),
                )
            ob = obp.tile([128, 512], BF16, name=f"ob{r}_{i}", tag="ob")
            nc.vector.tensor_copy(ob, ps)
            nc.sync.dma_start(
                out=dram["out"].ap()[i * 128:(i + 1) * 128, ng * 512:(ng + 1) * 512],
                in_=ob,
            )


def _emit_body(nc, c, P, pools, dram, rep):
    # projections for t-groups 0,1 -> attention q-cols 0:1024 (all heads)
    # -> projections 2,3 -> out-proj rows 0:1024 -> attention q-cols
    # 1024:2048 -> out-proj rows 1024:2048. This keeps the scalar engine's
    # exp stream (the phase-2 floor) running while the PE does projection
    # and out-projection work.
    _emit_proj_tg(nc, c, P, pools, dram, rep, 0)
    _emit_proj_tg(nc, c, P, pools, dram, rep, 1)
    for h in range(HPC):
        _emit_head_gpair(nc, c, P, pools, rep, h, (0, 1))
    _emit_proj_tg(nc, c, P, pools, dram, rep, 2)
    _emit_proj_tg(nc, c, P, pools, dram, rep, 3)
    _emit_wo(nc, c, P, pools, dram, rep, range(0, 8))
    for h in range(HPC):
        _emit_head_gpair(nc, c, P, pools, rep, h, (2, 3))
    _emit_wo(nc, c, P, pools, dram, rep, range(8, 16))


def build(reps=1):
    nc = bacc.Bacc("TRN2", target_bir_lowering=False, num_devices=NCORES)
    dram = {
        "xT": nc.dram_tensor("xT", [128, ND, T], BF16, kind="ExternalInput"),
        "xT8": nc.dram_tensor("xT8", [128, 4, 2, T], F8, kind="ExternalInput"),
        "wq8": nc.dram_tensor("wq8", [128, 4, 2, JS], F8, kind="ExternalInput"),
        "wk8": nc.dram_tensor("wk8", [128, 4, 2, JS], F8, kind="ExternalInput"),
        "wq8lo": nc.dram_tensor("wq8lo", [128, 4, 2, JS], F8, kind="ExternalInput"),
        "wk8lo": nc.dram_tensor("wk8lo", [128, 4, 2, JS], F8, kind="ExternalInput"),
        "wv": nc.dram_tensor("wv", [128, ND, JS], BF16, kind="ExternalInput"),
        "wo": nc.dram_tensor("wo", [128, 2, D], BF16, kind="ExternalInput"),
        "bq": nc.dram_tensor("bq", [128, 2], F32, kind="ExternalInput"),
        "bk": nc.dram_tensor("bk", [128, 2], F32, kind="ExternalInput"),
        "bv": nc.dram_tensor("bv", [JS], F32, kind="ExternalInput"),
        "out": nc.dram_tensor("out", [T, D], BF16, kind="ExternalOutput"),
    }
    with tile.TileContext(nc) as tc:
        with (
            tc.tile_pool(name="consts", bufs=1) as consts,
            tc.tile_pool(name="persist", bufs=1) as persistp,
            tc.tile_pool(name="xt", bufs=3) as xp,
            tc.tile_pool(name="x8", bufs=3) as x8p,
            tc.tile_pool(name="psP", bufs=2, space="PSUM") as psP,
            tc.tile_pool(name="psS", bufs=2, space="PSUM") as psS,
            tc.tile_pool(name="psA", bufs=2, space="PSUM") as psA,
            tc.tile_pool(name="es", bufs=6) as esb,
            tc.tile_pool(name="nrm", bufs=3) as nrm,
            tc.tile_pool(name="ob", bufs=4) as obp,
        ):
            c = _emit_consts(nc, consts, dram)
            QKDT = F8 if SCORES_DR else BF16
            P = {}
            for par in range(2):
                P[par] = {
                    "qT": persistp.tile([128, 2, T], QKDT, name=f"qT_{par}"),
                    "kT": persistp.tile([128, 2, T], QKDT, name=f"kT_{par}"),
                    "vv": persistp.tile(
                        [128, HPC, NT, HD + 2], BF16, name=f"vv_{par}"
                    ),
                    "oT": persistp.tile([128, 2, T], BF16, name=f"oT_{par}"),
                }
                # denominator row: 65th column of v is the constant 1
                nc.gpsimd.memset(P[par]["vv"][:, :, :, HD:HD + 1], 1.0)
            pools = {
                "xt": xp, "x8": x8p, "psP": psP, "psS": psS, "psA": psA,
                "es": esb, "nrm": nrm, "ob": obp,
            }
            for rep in range(reps):
                _emit_body(nc, c, P, pools, dram, rep)
    nc.compile()
    return nc


def _prep_core(x_b, wq, bq, wk, bk, wv, bv, wo, js):
    """Host-side shard + relayout + cast for one core."""
    f32 = np.float32
    xT = np.ascontiguousarray(x_b.T)  # [D, T], row d = dc*128+p
    xTb = np.ascontiguousarray(
        xT.reshape(ND, 128, T).transpose(1, 0, 2).astype(NP_BF16)
    )
    xT8 = np.ascontiguousarray(
        xT.reshape(4, 2, 128, T).transpose(2, 0, 1, 3).astype(NP_F8)
    )

    def qk_w(w):
        wp = (WS * w[:, js]).astype(f32)
        if SCORES_DR:
            wp = wp[:, PERM]
        hi = wp.astype(NP_F8)
        lo = (wp - hi.astype(f32)).astype(NP_F8)
        def lay(a):
            return np.ascontiguousarray(
                a.reshape(4, 2, 128, JS).transpose(2, 0, 1, 3)
            )
        return lay(hi), lay(lo)

    def qk_b(b):
        bp = (WS * b[js]).astype(f32)
        if SCORES_DR:
            bp = bp[PERM]
        return np.ascontiguousarray(bp.reshape(2, 128).T)

    wvc = np.ascontiguousarray(
        wv[:, js].reshape(ND, 128, JS).transpose(1, 0, 2).astype(NP_BF16)
    )
    woc = np.ascontiguousarray(
        wo[js, :].reshape(2, 2, HD, D).transpose(1, 2, 0, 3)
        .reshape(128, 2, D).astype(NP_BF16)
    )
    wq8, wq8lo = qk_w(wq)
    wk8, wk8lo = qk_w(wk)
    return {
        "xT": xTb,
        "xT8": xT8,
        "wq8": wq8,
        "wq8lo": wq8lo,
        "wk8": wk8,
        "wk8lo": wk8lo,
        "wv": wvc,
        "wo": woc,
        "bq": qk_b(bq),
        "bk": qk_b(bk),
        "bv": np.ascontiguousarray(bv[js].astype(f32)),
    }


def _in_maps(inputs):
    f32 = np.float32
    x = np.asarray(inputs["x"], f32)
    wq = np.asarray(inputs["wq"], f32)
    bq = np.asarray(inputs["bq"], f32)
    wk = np.asarray(inputs["wk"], f32)
    bk = np.asarray(inputs["bk"], f32)
    wv = np.asarray(inputs["wv"], f32)
    bv = np.asarray(inputs["bv"], f32)
    wo = np.asarray(inputs["wo"], f32)
    maps = []
    for cc in range(NCORES):
        b, g = cc // HPC, cc % HPC
        js = slice(g * JS, (g + 1) * JS)
        maps.append(_prep_core(x[b], wq, bq, wk, bk, wv, bv, wo, js))
    return maps


def kernel(**inputs) -> np.ndarray:
    from concourse.bass_utils import run_bass_kernel_spmd

    if "nc" not in _CACHE:
        _CACHE["nc"] = build()
    nc = _CACHE["nc"]
    maps = _in_maps(inputs)
    res = run_bass_kernel_spmd(nc, maps, core_ids=list(range(NCORES)))
    out = np.zeros((B, T, D), dtype=np.float32)
    for cc in range(NCORES):
        out[cc // HPC] += np.asarray(res.results[cc]["out"], dtype=np.float32)
    out += np.asarray(inputs["bo"], np.float32)[None, None, :]
    return out
